# revision 27
# baseline (speedup 1.0000x reference)
"""BayesianBlock (LN -> reparameterized linear -> exact GELU -> residual) on 8 trn2 cores.

Sharding: tokens (8192) split 2x across cores, out-features (4096) split 4x.
Each core's inputs have the hidden axis rolled by -q*1024 (q = o-quarter index)
so the residual columns are always x[:, 0:1024] -- LayerNorm and the hidden
contraction are invariant to a consistent permutation of the hidden axis, so
the SPMD program is identical across cores.

v2 design ("LN folded into the matmul epilogue", all-bf16 PE pipeline):
  - Host pre-transposes W^T slices (mu/rho/eps stacked [3, H, O_SH]) and
    converts x / W / smalls to bf16. Layout-only + quantization; the
    reparameterization (softplus combine) stays on device.
  - No PE transposes at all: x^T tiles come straight from DRAM via the
    XBAR dma_start_transpose (14ns per 16x128 tile); W^T is DMA'd already
    transposed.
  - The matmul runs on RAW x^T (pre-LN). LayerNorm is exact-folded into the
    epilogue:  y = r_t * (x^T W'')[t,o] - r_t*mu_t*S[o] + bias_hat[o]
    where W'' = gamma (.) W,  S = colsum(W''),  bias_hat = b + beta.W.
    Epilogue: corr = S*mu_t (ACT), corr2 = b_hat*(1/r_t) - corr (Pool),
    yp += corr2 (DVE, PSUM), out = GELU(yp * r_t) (ACT scale), + residual.
  - S and beta.W come from one fused PE sweep over wt with a [ones | beta]
    stationary (partitions 0:64 get S, 64:128 get beta.W), broadcast to all
    partitions via two small DMA hops.
  - Startup: W DMA is k-major on the sync queue; the first 3 token tiles'
    matmuls run k-major interleaved with the sweep so PE chews each wt
    k-slice as it lands. Remaining 29 tiles run tile-major at full speed.
"""

import numpy as np
import ml_dtypes

import concourse.bass as bass
import concourse.mybir as mybir
import concourse.tile as tile
from concourse import bacc, bass_utils

F32 = mybir.dt.float32
BF16 = mybir.dt.bfloat16
AF = mybir.ActivationFunctionType
ALU = mybir.AluOpType

B, S, H = 4, 2048, 4096
NTOK = B * S                  # 8192
N_CORES = 8
TOK_SPLIT, O_SPLIT = 2, 4
TOK_SH = NTOK // TOK_SPLIT    # 4096 tokens per core
O_SH = H // O_SPLIT           # 1024 out features per core
LN_EPS = 1e-5

TOK_TILES = TOK_SH // 128     # 32
K_TILES = H // 128            # 32
O_PANELS = O_SH // 512        # 2
N_EARLY = 3                   # token tiles processed k-major during W stream
SM_LEN = 2 * H + 3 * O_SH     # gamma | beta | b_mu | b_rho | eps_b

_CACHED = {}


def _patch_act_tables():
    """Make exp/ln resolve to the single table containing both, so the
    greedy act-table chooser doesn't swap tables between Exp and Ln."""
    if getattr(bacc, "_act_tables_patched", False):
        return
    orig = bacc.get_activation_tables

    def patched(module_arch):
        tabs = orig(module_arch)
        exp = mybir.ActivationFunctionType.Exp
        ln = mybir.ActivationFunctionType.Ln
        for name, funcs in tabs.items():
            if name != "natural_log_exp_and_others":
                funcs.discard(exp)
                funcs.discard(ln)
        return tabs

    bacc.get_activation_tables = patched
    bacc._act_tables_patched = True


def build_nc():
    import os

    n_repeat = int(os.environ.get("K_REPEAT", "1"))
    _patch_act_tables()
    nc = bacc.Bacc("TRN2", target_bir_lowering=False, debug=False, num_devices=1)
    x = nc.dram_tensor("x", [TOK_SH, H], BF16, kind="ExternalInput").ap()
    wpack = nc.dram_tensor("wpack", [3, H, O_SH], BF16, kind="ExternalInput").ap()
    smalls = nc.dram_tensor("smalls", [SM_LEN], BF16, kind="ExternalInput").ap()
    out = nc.dram_tensor("out", [TOK_SH, O_SH], F32, kind="ExternalOutput").ap()

    with tile.TileContext(nc) as tc:
        with (
            tc.tile_pool(name="persist", bufs=1) as persist,
            tc.tile_pool(name="wst", bufs=2) as wst,
            tc.tile_pool(name="xp", bufs=1) as xp,
            tc.tile_pool(name="htp", bufs=1) as htp,
            tc.tile_pool(name="op", bufs=2) as op_pool,
            tc.tile_pool(name="stp", bufs=2) as stp,
            tc.tile_pool(name="yps", bufs=1, space="PSUM") as yps,
        ):
            # ---------------- prologue ----------------
            # gamma/beta column tiles [128, K_TILES]: [p, k] = g[k*128 + p],
            # via XBAR transpose of the natural [K_TILES, 128] rows.
            gb16 = wst.tile([128, 2 * K_TILES], BF16, tag="gb16", name="gb16", bufs=1)
            nc.sync.dma_start_transpose(
                out=gb16[:, 0:K_TILES], in_=smalls[0:H].rearrange("(k p) -> k p", p=128)
            )
            nc.sync.dma_start_transpose(
                out=gb16[:, K_TILES : 2 * K_TILES],
                in_=smalls[H : 2 * H].rearrange("(k p) -> k p", p=128),
            )
            gb = persist.tile([128, 2 * K_TILES], F32)
            nc.vector.tensor_copy(out=gb, in_=gb16)
            gamma_col = gb[:, 0:K_TILES]
            beta_col = gb[:, K_TILES : 2 * K_TILES]

            # b_hat tile (filled after the W loop so the b-chain's ACT ops
            # don't sit at the head of the ACT queue and delay W softplus)
            b_hat = persist.tile([128, O_SH], F32)

            def emit_b_chain():
                t_bmu = wst.tile([128, O_SH], BF16, tag="bmu", name="bmu", bufs=1)
                t_brho = wst.tile([128, O_SH], BF16, tag="brho", name="brho", bufs=1)
                t_beps = wst.tile([128, O_SH], BF16, tag="beps", name="beps", bufs=1)
                sm_b = 2 * H
                nc.sync.dma_start(
                    out=t_bmu, in_=smalls[sm_b : sm_b + O_SH].partition_broadcast(128)
                )
                nc.sync.dma_start(
                    out=t_brho,
                    in_=smalls[sm_b + O_SH : sm_b + 2 * O_SH].partition_broadcast(128),
                )
                nc.sync.dma_start(
                    out=t_beps,
                    in_=smalls[sm_b + 2 * O_SH : sm_b + 3 * O_SH].partition_broadcast(128),
                )
                nc.scalar.activation(out=t_brho, in_=t_brho, func=AF.Exp)
                nc.scalar.activation(out=t_brho, in_=t_brho, func=AF.Ln, bias=1.0)
                nc.vector.tensor_mul(out=t_brho, in0=t_brho, in1=t_beps)
                nc.vector.tensor_add(out=b_hat, in0=t_brho, in1=t_bmu)

            # resident W''^T, bf16: [128, k, o] = gamma[k*128+p] * W[o, k*128+p]
            wt = persist.tile([128, K_TILES, O_SH], BF16)
            ones_col = persist.tile([128, 64], BF16)
            nc.gpsimd.memset(ones_col, 1.0)
            # ones stationary for the K=1 PE broadcast of the sweep rows
            ones1 = persist.tile([1, 128], F32)
            nc.gpsimd.memset(ones1, 1.0)

            # ---------------- x/xT for the early tiles (scalar queue) -------
            def tsl(i):
                return slice((i % TOK_TILES) * 128, (i % TOK_TILES) * 128 + 128)

            xts, hts, stats_out = {}, {}, {}

            def emit_ht_dma(i):
                ht = htp.tile([128, K_TILES, 128], BF16, tag="ht", name=f"ht{i}", bufs=4)
                nc.sync.dma_start_transpose(out=ht, in_=x[tsl(i), :])
                hts[i] = ht

            def emit_xt_dma(i):
                xt = xp.tile([128, H], BF16, tag="x", name=f"x{i}", bufs=5)
                nc.sync.dma_start(out=xt, in_=x[tsl(i), :])
                xts[i] = xt

            def emit_x_dma(i):
                emit_xt_dma(i)
                emit_ht_dma(i)

            def emit_stats(i):
                xt = xts[i]
                st = stp.tile([128, H // 512, nc.vector.BN_STATS_DIM], F32, tag="st", name=f"st{i}")
                xg = xt[:].rearrange("p (s f) -> p s f", f=512)
                for sgi in range(H // 512):
                    nc.vector.bn_stats(out=st[:, sgi, :], in_=xg[:, sgi, :])
                mv = stp.tile([128, nc.vector.BN_AGGR_DIM], F32, tag="mv", name=f"mv{i}", bufs=5)
                nc.vector.bn_aggr(out=mv, in_=st[:])

                # rstd = 1/sqrt(var+eps) via Newton (seed 0.5+0.5/u, 1 iter)
                u = stp.tile([128, 1], F32, tag="u", name=f"u{i}")
                nc.vector.tensor_scalar_add(out=u, in0=mv[:, 1:2], scalar1=LN_EPS)
                rstd = stp.tile([128, 1], F32, tag="rstd", name=f"rstd{i}", bufs=5)
                nc.vector.reciprocal(out=rstd, in_=u)
                nc.vector.tensor_scalar(
                    out=rstd, in0=rstd, scalar1=0.5, scalar2=0.5, op0=ALU.mult, op1=ALU.add
                )
                t1 = stp.tile([128, 1], F32, tag="t1", name=f"t1{i}")
                nc.vector.tensor_mul(out=t1, in0=rstd, in1=rstd)
                nc.vector.tensor_mul(out=t1, in0=t1, in1=u)
                nc.vector.tensor_scalar(
                    out=t1, in0=t1, scalar1=-0.5, scalar2=1.5, op0=ALU.mult, op1=ALU.add
                )
                nc.vector.tensor_mul(out=rstd, in0=rstd, in1=t1)
                # 1/rstd = u * rstd (since rstd ~= 1/sqrt(u))
                invr = stp.tile([128, 1], F32, tag="invr", name=f"invr{i}", bufs=5)
                nc.vector.tensor_mul(out=invr, in0=u, in1=rstd)
                stats_out[i] = (mv, rstd, invr)

            def emit_epilogue(i, yp_panels):
                mv, rstd, invr = stats_out[i]
                xt = xts[i]
                o_t = op_pool.tile([128, O_SH], F32, tag="o", name=f"o{i}")
                for opi in range(O_PANELS):
                    osl = slice(opi * 512, (opi + 1) * 512)
                    yp = yp_panels[opi]
                    corr = op_pool.tile([128, 512], F32, tag="corr", name=f"corr{i}_{opi}", bufs=3)
                    # corr = S * mu_t   (ACT Copy with per-partition scale,
                    # S read straight from its resident PSUM bank)
                    nc.scalar.mul(corr, s_psum[opi][:], mv[:, 0:1])
                    # corr2 = b_hat * (1/r_t) - corr   (Pool)
                    corr2 = op_pool.tile([128, 512], F32, tag="corr2", name=f"corr2{i}_{opi}", bufs=3)
                    nc.gpsimd.tensor_scalar_mul(out=corr2, in0=b_hat[:, osl], scalar1=invr[:])
                    nc.gpsimd.tensor_sub(out=corr2, in0=corr2, in1=corr)
                    # yp += corr2 ; out = GELU(yp * r_t)
                    nc.vector.tensor_add(out=yp, in0=yp, in1=corr2)
                    nc.scalar.activation(
                        out=o_t[:, osl], in_=yp, func=AF.Gelu, scale=rstd[:]
                    )
                # residual (pristine bf16 x columns 0:O_SH) and store
                nc.vector.tensor_add(out=o_t, in0=o_t, in1=xt[:, 0:O_SH])
                nc.scalar.dma_start(out=out[tsl(i), :], in_=o_t)

            # hT for the early tiles first (they gate the k-major rounds);
            # their x tiles and stats are woven into the W stream below.
            for i in range(N_EARLY):
                emit_ht_dma(i)

            # ---------------- W phase (k-major) + early tiles ----------------
            early_psum = [
                [
                    yps.tile([128, 512], F32, tag=f"y{opi}", name=f"y{i}_{opi}", bufs=3)
                    for opi in range(O_PANELS)
                ]
                for i in range(N_EARLY)
            ]
            sweep_psum = [
                yps.tile([128, 512], F32, tag=f"sw{opi}", name=f"sw{opi}", bufs=1)
                for opi in range(O_PANELS)
            ]
            for k in range(K_TILES):
                # weave x-tile DMAs and their stats into the W stream: early
                # tiles' x at k=2/6/10, tiles 3-4 fully prefetched at k=14/20
                # so the token loop has work the moment PSUM banks free up
                if k in (2, 6, 10):
                    emit_xt_dma((k - 2) // 4)
                elif k == 14:
                    emit_x_dma(N_EARLY)
                elif k == 20:
                    emit_x_dma(N_EARLY + 1)
                if k in (5, 9, 13):
                    emit_stats((k - 5) // 4)
                t_mu = wst.tile([128, O_SH], BF16, tag="wmu", name=f"wmu{k}", bufs=3)
                t_rho = wst.tile([128, O_SH], BF16, tag="wrho", name=f"wrho{k}", bufs=3)
                t_eps = wst.tile([128, O_SH], BF16, tag="weps", name=f"weps{k}", bufs=3)
                ksl = slice(k * 128, (k + 1) * 128)
                nc.sync.dma_start(out=t_mu, in_=wpack[0, ksl, :])
                nc.sync.dma_start(out=t_rho, in_=wpack[1, ksl, :])
                nc.sync.dma_start(out=t_eps, in_=wpack[2, ksl, :])
                # softplus(rho) = Ln(Exp(rho) + 1) on ACT
                nc.scalar.activation(out=t_rho, in_=t_rho, func=AF.Exp)
                nc.scalar.activation(out=t_rho, in_=t_rho, func=AF.Ln, bias=1.0)
                # W'' = gamma * (mu + sp * eps) on DVE (bf16 2x)
                nc.vector.tensor_mul(out=t_rho, in0=t_rho, in1=t_eps)
                nc.vector.tensor_add(out=t_mu, in0=t_mu, in1=t_rho)
                nc.vector.tensor_scalar_mul(
                    out=wt[:, k, :], in0=t_mu, scalar1=gamma_col[:, k : k + 1]
                )

                # fused sweep stationary: cols 0:64 = ones -> S on partitions
                # 0:64; cols 64:128 = beta[k*128+p] -> beta.W on 64:128
                st2 = wst.tile([128, 128], BF16, tag="st2", name=f"st2{k}")
                nc.gpsimd.tensor_copy(out=st2[:, 0:64], in_=ones_col)
                nc.gpsimd.tensor_scalar_mul(
                    out=st2[:, 64:128], in0=ones_col, scalar1=beta_col[:, k : k + 1]
                )
                # k-outer, panel-inner: one Ldweights per stationary, two matmuls
                for opi in range(O_PANELS):
                    osl = slice(opi * 512, (opi + 1) * 512)
                    nc.tensor.matmul(
                        sweep_psum[opi], st2[:], wt[:, k, osl],
                        start=(k == 0), stop=(k == K_TILES - 1),
                    )
                for i in range(N_EARLY):
                    for opi in range(O_PANELS):
                        osl = slice(opi * 512, (opi + 1) * 512)
                        nc.tensor.matmul(
                            early_psum[i][opi], hts[i][:, k, :], wt[:, k, osl],
                            start=(k == 0), stop=(k == K_TILES - 1),
                        )

            emit_b_chain()

            # broadcast sweep rows to all partitions without leaving the
            # core: evict the two rows to SBUF, then K=2 PE matmuls against
            # the selector stationary. S stays resident in the sweep PSUM
            # banks (read by every epilogue's corr op); bW folds into b_hat.
            brow_s = persist.tile([1, O_SH], F32)
            brow_b = persist.tile([1, O_SH], F32)
            s_psum = []
            for opi in range(O_PANELS):
                osl = slice(opi * 512, (opi + 1) * 512)
                nc.vector.tensor_copy(out=brow_s[0:1, osl], in_=sweep_psum[opi][0:1, :])
                nc.vector.tensor_copy(out=brow_b[0:1, osl], in_=sweep_psum[opi][64:65, :])
            for opi in range(O_PANELS):
                osl = slice(opi * 512, (opi + 1) * 512)
                bw_ps = yps.tile([128, 512], F32, tag=f"sw{opi}", name=f"bwps{opi}", bufs=1)
                nc.tensor.matmul(bw_ps, ones1[:], brow_b[0:1, osl])
                nc.vector.tensor_add(out=b_hat[:, osl], in0=b_hat[:, osl], in1=bw_ps)
                s_ps = yps.tile([128, 512], F32, tag=f"sw{opi}", name=f"sps{opi}", bufs=1)
                nc.tensor.matmul(s_ps, ones1[:], brow_s[0:1, osl])
                s_psum.append(s_ps)

            for i in range(N_EARLY):
                emit_epilogue(i, early_psum[i])

            # ---------------- token loop (tile-major) ----------------
            for it0 in range(N_EARLY, TOK_TILES * n_repeat):
                if it0 not in xts:
                    emit_x_dma(it0)
                emit_stats(it0)
                yp_panels = [
                    yps.tile([128, 512], F32, tag=f"y{opi}", name=f"y{it0}_{opi}", bufs=3)
                    for opi in range(O_PANELS)
                ]
                for k in range(K_TILES):
                    for opi in range(O_PANELS):
                        osl = slice(opi * 512, (opi + 1) * 512)
                        nc.tensor.matmul(
                            yp_panels[opi], hts[it0][:, k, :], wt[:, k, osl],
                            start=(k == 0), stop=(k == K_TILES - 1),
                        )
                emit_epilogue(it0, yp_panels)

    nc.compile()
    return nc


def prepare_in_maps(x, ln_gamma, ln_beta, w_mu, w_rho, b_mu, b_rho, eps_w, eps_b):
    BF = ml_dtypes.bfloat16
    x_flat = np.asarray(x, dtype=np.float32).reshape(NTOK, H)
    ln_gamma = np.asarray(ln_gamma, dtype=np.float32)
    ln_beta = np.asarray(ln_beta, dtype=np.float32)
    b_mu = np.asarray(b_mu, dtype=np.float32)
    b_rho = np.asarray(b_rho, dtype=np.float32)
    eps_b = np.asarray(eps_b, dtype=np.float32)
    # global transposes once: [h, o_full]
    wT = {k: np.ascontiguousarray(np.asarray(m, dtype=np.float32).T)
          for k, m in (("mu", w_mu), ("rho", w_rho), ("eps", eps_w))}

    in_maps = []
    for c in range(N_CORES):
        th, q = divmod(c, O_SPLIT)
        r = q * O_SH
        osl = slice(q * O_SH, (q + 1) * O_SH)
        xs = x_flat[th * TOK_SH : (th + 1) * TOK_SH]
        wpack = np.stack(
            [np.roll(wT[k][:, osl], -r, axis=0).astype(BF) for k in ("mu", "rho", "eps")]
        )
        smalls = np.concatenate(
            [np.roll(ln_gamma, -r), np.roll(ln_beta, -r), b_mu[osl], b_rho[osl], eps_b[osl]]
        ).astype(BF)
        in_maps.append(
            {
                "x": np.ascontiguousarray(np.roll(xs, -r, axis=1).astype(BF)),
                "wpack": np.ascontiguousarray(wpack),
                "smalls": smalls,
            }
        )
    return in_maps


def assemble_out(results):
    out_full = np.empty((NTOK, H), dtype=np.float32)
    for c in range(N_CORES):
        th, q = divmod(c, O_SPLIT)
        out_full[
            th * TOK_SH : (th + 1) * TOK_SH, q * O_SH : (q + 1) * O_SH
        ] = results[c]["out"]
    return out_full.reshape(B, S, H)


def kernel(**inputs) -> np.ndarray:
    if "nc" not in _CACHED:
        _CACHED["nc"] = build_nc()
    nc = _CACHED["nc"]
    in_maps = prepare_in_maps(**inputs)
    res = bass_utils.run_bass_kernel_spmd(
        nc, in_maps, core_ids=list(range(N_CORES)), trace=False
    )
    return assemble_out(res.results)
